# revision 2
# baseline (speedup 1.0000x reference)
"""MultiHeadAttention Trainium2 kernel (8-core SPMD, collective-free).

Problem: B=4, T=2048, E=1024, H=16, D=64 multi-head self-attention
(torch-style Linear projections, softmax over keys, output projection).

Sharding: core c handles batch b=c//2 and query-token half qh=c%2
(1024 query tokens) with ALL 16 heads local, so the output projection
contracts over the full E=1024 attention features with no cross-core
communication at all (the collective path through fake_nrt costs ~60ms
per AllGather, ~250ms/iter -- removing it is worth the 2x duplicated
K/V projection work, +55us of PE time).  K and V cover all 2048 keys;
each core's xt is host-side PERMUTED so its own query half comes first
(tokens 0:1024) -- softmax is permutation-invariant over keys, so the
same SPMD program works on every core.

Device pipeline per core (storage bf16, all accumulation fp32):
  - bootstrap: q/k projection fillers + V projection (PE-dense; ACT has
    less total work than PE so idling ACT here costs nothing),
  - 32 weave units (16 heads x 2 query blocks of 512), each unit with
    IN-UNIT software pipelining: per kc-pair group g emit 2 score
    matmuls (scores^T [128 keys, 512 q], K=64, head-parity
    tile_position), one exp ACT [128,1024] (scale=1/8 fused, logits
    bounded so no max subtraction), and the PV matmuls of group g-1
    reading a 4-slot exp ring (8KB instead of a 32KB per-unit double
    buffer -- that is what lets 16 heads of K/V/Q/out state fit SBUF),
  - PV lhsT = [v_h | 1]: psum row 64 accumulates the softmax
    denominator for free; K=1 float32r ones-matmul broadcasts it across
    partitions; DVE reciprocal + multiply normalizes,
  - odd heads' outputs are partition-shifted 0:64 -> 64:128 with a
    small SBUF->SBUF DMA so outT keeps a feature-major layout,
  - output projection per 128-token chunk as weave fillers (contracts
    all 8 feature chunks of outT against resident Wo^T).
"""

import os
import sys
from contextlib import ExitStack

import numpy as np
import ml_dtypes

for _p in ("/opt/trn_rl_repo", "/root/.axon_site/_ro/trn_rl_repo"):
    if os.path.isdir(_p) and _p not in sys.path:
        sys.path.insert(0, _p)

import concourse.bass as bass  # noqa: E402,F401
from concourse import bacc  # noqa: E402
import concourse.tile as tile  # noqa: E402
from concourse import mybir  # noqa: E402
from concourse.bass_utils import run_bass_kernel_spmd  # noqa: E402

# ---- problem constants (hardcoded; kernel.py must be self-contained) ----
B, T, E, H, D = 4, 2048, 1024, 16, 64
P = 128
NCORES = 8
QT = 1024            # local query tokens per core
EC = E // P          # 8 contraction chunks for projections
HP = H // 2          # 8 head-pair feature chunks (128 rows = 2 heads)
KC = T // P          # 16 key-token chunks
QB = QT // 512       # 2 query blocks of 512
TC = QT // P         # 8 output token chunks of 128
NG = KC // 2         # 8 kc-pair score groups per unit

BF = mybir.dt.bfloat16
F32 = mybir.dt.float32
F32R = mybir.dt.float32r
AF = mybir.ActivationFunctionType
ALU = mybir.AluOpType

SECTIONS = []        # (name, first_instruction_index) markers for profiling
USE_TILE_POS = os.environ.get("KERNEL_NO_TILEPOS", "0") != "1"
REPEAT = int(os.environ.get("KERNEL_REPEAT", "1"))


def build_program():
    nc = bacc.Bacc("TRN2", target_bir_lowering=False, debug=False,
                   num_devices=NCORES)

    def mark(name):
        SECTIONS.append((name, len(nc.inst_map)))

    xt_d = nc.dram_tensor("xt", [EC, P, T], BF, kind="ExternalInput").ap()
    wqt_d = nc.dram_tensor("wqt", [EC, P, E], BF, kind="ExternalInput").ap()
    wkt_d = nc.dram_tensor("wkt", [EC, P, E], BF, kind="ExternalInput").ap()
    wvt_d = nc.dram_tensor("wvt", [EC, P, E], BF, kind="ExternalInput").ap()
    wot_d = nc.dram_tensor("wot", [EC, P, E], BF, kind="ExternalInput").ap()
    bq_d = nc.dram_tensor("bq", [HP, P], F32, kind="ExternalInput").ap()
    bk_d = nc.dram_tensor("bk", [HP, P], F32, kind="ExternalInput").ap()
    bvb_d = nc.dram_tensor("bvb", [P, E], F32, kind="ExternalInput").ap()
    bob_d = nc.dram_tensor("bob", [P, E], F32, kind="ExternalInput").ap()
    ones_d = nc.dram_tensor("ones", [P, 64], F32R, kind="ExternalInput").ap()
    out_d = nc.dram_tensor("out", [TC, P, E], F32,
                           kind="ExternalOutput").ap()

    with tile.TileContext(nc) as tc, ExitStack() as ctx:
        persist = ctx.enter_context(tc.tile_pool(name="persist", bufs=1))
        wq_pool = ctx.enter_context(tc.tile_pool(name="wq", bufs=2))
        wv_pool = ctx.enter_context(tc.tile_pool(name="wv", bufs=2))
        ehp = ctx.enter_context(tc.tile_pool(name="ehp", bufs=4))
        small = ctx.enter_context(tc.tile_pool(name="small", bufs=2))
        otmp_pool = ctx.enter_context(tc.tile_pool(name="otmp", bufs=1))
        fin_pool = ctx.enter_context(tc.tile_pool(name="finp", bufs=2))
        psc = ctx.enter_context(tc.tile_pool(name="psc", bufs=3, space="PSUM"))
        ppv = ctx.enter_context(tc.tile_pool(name="ppv", bufs=2, space="PSUM"))

        def sc_slot():
            return psc.tile([P, 2, 512], F32, tag="sc", name="sc")

        # ---------------- persistent SBUF tensors ----------------
        xt_sb = persist.tile([P, EC, T], BF, tag="xt")          # 32K
        bq_sb = persist.tile([P, HP], F32, tag="bq")
        bk_sb = persist.tile([P, HP], F32, tag="bk")
        bvb_sb = persist.tile([P, E], F32, tag="bvb")           # 4K
        bob_sb = persist.tile([P, E], F32, tag="bob")           # 4K
        wot_sb = persist.tile([P, EC, E], BF, tag="wot")        # 16K
        qt_sb = persist.tile([P, HP, QT], BF, tag="qt")         # 16K
        kt_sb = persist.tile([P, HP, T], BF, tag="kt")          # 32K
        vaug = persist.tile([P, KC, H * 65], BF, tag="vaug")    # 32.5K
        outT = persist.tile([P, HP, QT], BF, tag="outT")        # 16K
        ones_sb = persist.tile([P, 64], F32R, tag="ones")

        mark('in_dma')
        # ---------------- input DMAs ----------------
        for ec in range(EC):
            nc.sync.dma_start(xt_sb[:, ec, :], xt_d[ec])
        nc.sync.dma_start(bq_sb[:], bq_d.rearrange("f p -> p f"))
        nc.sync.dma_start(bk_sb[:], bk_d.rearrange("f p -> p f"))
        nc.sync.dma_start(bvb_sb[:], bvb_d)
        nc.sync.dma_start(ones_sb[:], ones_d)
        vaug_h = vaug.rearrange("p k (h c) -> p k h c", c=65)
        for h in range(H):
            nc.gpsimd.memset(vaug_h[:, :, h, 64:65], 1.0)

        # units: query-block major so each block's output projection can
        # start as early as possible
        UNITS = [(h, qb) for qb in range(QB) for h in range(H)]

        filler = []

        def drain_filler(n=1):
            for _ in range(n):
                if not filler:
                    return
                filler.pop(0)()

        def queue_qk(fc):
            """qT/kT projection for feature chunk fc (head pair) as
            fillers."""
            fs = slice(fc * P, (fc + 1) * P)
            box = {}

            def dma_w():
                box["wqf"] = wq_pool.tile([P, EC, P], BF, tag="wqf",
                                          name="wqf")
                box["wkf"] = wq_pool.tile([P, EC, P], BF, tag="wkf",
                                          name="wkf")
                for ec in range(EC):
                    nc.sync.dma_start(box["wqf"][:, ec, :], wqt_d[ec, :, fs])
                    nc.sync.dma_start(box["wkf"][:, ec, :], wkt_d[ec, :, fs])
            filler.append(dma_w)

            def group(kind, tp):
                """One [P, 1024] token group of q or k projection."""
                w_key = "wqf" if kind == "q" else "wkf"
                dest = qt_sb if kind == "q" else kt_sb
                bias = bq_sb if kind == "q" else bk_sb
                ps_box = {}

                def mms(lo, hi):
                    def _f():
                        if "ps" not in ps_box:
                            ps_box["ps"] = sc_slot()
                        ps = ps_box["ps"]
                        for i in range(2):
                            tb = 2 * tp + i
                            for ec in range(lo, hi):
                                nc.tensor.matmul(
                                    ps[:, i, :], lhsT=box[w_key][:, ec, :],
                                    rhs=xt_sb[:, ec, tb * 512:(tb + 1) * 512],
                                    start=(ec == 0), stop=(ec == EC - 1),
                                )
                    return _f
                filler.append(mms(0, 4))
                filler.append(mms(4, 8))

                def evac():
                    nc.vector.tensor_scalar_add(
                        dest[:, fc, tp * 1024:(tp + 1) * 1024],
                        ps_box["ps"].rearrange("p a b -> p (a b)"),
                        bias[:, fc: fc + 1],
                    )
                filler.append(evac)
            group("q", 0)                    # q: 1024 local query tokens
            for tp in range(T // 1024):      # k: all 2048 key tokens
                group("k", tp)

        def queue_outproj(qb):
            """final[t, :] for the 4 token chunks of query block qb."""
            for tcl in range(4):
                tc_ = qb * 4 + tcl
                ps_box = {}

                def mms(lo, hi, tc_=tc_, ps_box=ps_box):
                    def _f():
                        if "ps" not in ps_box:
                            ps_box["ps"] = sc_slot()
                        ps = ps_box["ps"]
                        for fc in range(lo, hi):
                            for i in range(2):
                                nc.tensor.matmul(
                                    ps[:, i, :],
                                    lhsT=outT[:, fc, tc_ * P:(tc_ + 1) * P],
                                    rhs=wot_sb[:, fc, i * 512:(i + 1) * 512],
                                    start=(fc == 0), stop=(fc == EC - 1),
                                )
                    return _f
                filler.append(mms(0, 4))
                filler.append(mms(4, 8))

                def evac(tc_=tc_, ps_box=ps_box):
                    fin = fin_pool.tile([P, E], F32, tag="fin", name="fin")
                    nc.vector.tensor_tensor(
                        fin[:], ps_box["ps"].rearrange("p a b -> p (a b)"),
                        bob_sb[:], ALU.add)
                    nc.sync.dma_start(out_d[tc_], fin[:])
                filler.append(evac)

        def pv_finish(h, qb, po):
            hp, par = h // 2, h % 2
            qs = slice(qb * 512, (qb + 1) * 512)
            srb = small.tile([P, 512], F32R, tag="srb", name="srb")
            nc.vector.tensor_copy(srb[64:65, :], po[64:65, :])
            psR = sc_slot()
            nc.tensor.matmul(psR[0:64, 0, :], lhsT=ones_sb[64:65, :],
                             rhs=srb[64:65, :], start=True, stop=True)
            with nc.allow_low_precision(
                    reason="float32r is bit-identical fp32 storage"):
                nc.vector.reciprocal(srb[0:64, :], psR[0:64, 0, :])
            if par == 0:
                nc.vector.tensor_tensor(outT[0:64, hp, qs], po[0:64, :],
                                        srb[0:64, :], ALU.mult)
            else:
                ot = otmp_pool.tile([P, 512], BF, tag="ot", name="ot")
                nc.vector.tensor_tensor(ot[0:64, :], po[0:64, :],
                                        srb[0:64, :], ALU.mult)
                nc.sync.dma_start(outT[64:128, hp, qs], ot[0:64, :])

        finish_box = {}  # (h, qb, po) of the previous unit, not yet finished

        def weave_unit(ui):
            """Emit unit ui: per group g, 2 score matmuls + exp ACT, then
            the PV matmuls of group g-1 (in-unit lag via the exp ring);
            the previous unit's pv_finish lands at group 2 so its psR
            matmul never stalls PE on the DVE denominator copy."""
            h, qb = UNITS[ui]
            hp, par = h // 2, h % 2
            qs = slice(qb * 512, (qb + 1) * 512)
            rows = slice(0, 64) if par == 0 else slice(64, 128)
            tp = (dict(tile_position=(0, 0)) if par == 0 else
                  dict(tile_position=(64, 0))) if USE_TILE_POS else {}
            po = ppv.tile([P, 512], F32, tag="po", name="po")
            ring = [None] * NG
            for g in range(NG + 1):
                if g < NG:
                    ps2 = sc_slot()
                    for i in range(2):
                        kc = 2 * g + i
                        kslc = slice(kc * P, (kc + 1) * P)
                        nc.tensor.matmul(
                            ps2[:, i, :], lhsT=kt_sb[rows, hp, kslc],
                            rhs=qt_sb[rows, hp, qs],
                            start=True, stop=True, **tp,
                        )
                    eh = ehp.tile([P, 2, 512], BF, tag="eh", name="eh")
                    nc.scalar.activation(eh[:], ps2[:], AF.Exp, scale=0.125)
                    ring[g] = eh
                if g >= 1:
                    pg = g - 1
                    for i in range(2):
                        kc = 2 * pg + i
                        nc.tensor.matmul(
                            po[0:65, :], lhsT=vaug_h[:, kc, h, :],
                            rhs=ring[pg][:, i, :],
                            start=(kc == 0), stop=(kc == KC - 1),
                        )
                if g == 2 and finish_box:
                    pv_finish(*finish_box.pop("prev"))
                drain_filler(1)
            finish_box["prev"] = (h, qb, po)

        def emit_body():
            mark('proj')
            # bootstrap: q/k projections for the first chunks + the full
            # v projection -- PE-dense, ACT idle (ACT has spare budget)
            queue_qk(0)
            queue_qk(1)

            # ------------- v projection (token-major, +bv) -------------
            bvb_v = bvb_sb.rearrange("p (h d) -> p h d", d=D)
            for hf in range(2):          # feature half = heads hf*8..+8
                wvh = wv_pool.tile([P, EC, 512], BF, tag="wvh", name="wvh")
                for ec in range(EC):
                    nc.sync.dma_start(wvh[:, ec, :],
                                      wvt_d[ec, :, hf * 512:(hf + 1) * 512])
                for kp in range(KC // 2):
                    ps = sc_slot()
                    for i in range(2):
                        kc = 2 * kp + i
                        for ec in range(EC):
                            nc.tensor.matmul(
                                ps[:, i, :],
                                lhsT=xt_sb[:, ec, kc * P:(kc + 1) * P],
                                rhs=wvh[:, ec, :],
                                start=(ec == 0), stop=(ec == EC - 1),
                            )
                    nc.vector.tensor_tensor(
                        vaug_h[:, 2 * kp: 2 * kp + 2,
                               hf * 8:(hf + 1) * 8, 0:64],
                        ps.rearrange("p a (h d) -> p a h d", d=D),
                        bvb_v[:, None, hf * 8:(hf + 1) * 8, :]
                        .to_broadcast((P, 2, 8, D)),
                        ALU.add,
                    )
                    drain_filler(1)

            mark('attention')
            for ec in range(EC):
                nc.sync.dma_start(wot_sb[:, ec, :], wot_d[ec])
            nc.sync.dma_start(bob_sb[:], bob_d)

            for ui in range(len(UNITS)):
                if ui in (0, 2, 4, 6, 8, 10):   # qk(2..7) ahead of need
                    queue_qk(ui // 2 + 2)       # (chunk fc needed at ui=2fc)
                if ui == H + 1:                 # outT[:, :, qb0] complete
                    queue_outproj(0)
                weave_unit(ui)
            pv_finish(*finish_box.pop("prev"))
            queue_outproj(QB - 1)
            drain_filler(100)

        for _rep in range(REPEAT):
            emit_body()

        mark('tail')
    nc.compile()
    return nc


_NC = None


def _get_nc():
    global _NC
    if _NC is None:
        _NC = build_program()
    return _NC


def _prep_core_inputs(x, Wq, bq, Wk, bk, Wv, bv, Wo, bo):
    """Build the 8 per-core input dicts (host-side sharding)."""
    bf = ml_dtypes.bfloat16
    x = np.asarray(x, dtype=np.float32)
    Wq, Wk, Wv, Wo = (np.asarray(a, np.float32) for a in (Wq, Wk, Wv, Wo))
    bq, bk, bv, bo = (np.asarray(a, np.float32) for a in (bq, bk, bv, bo))
    ones_a = np.ones((P, 64), np.float32)

    # weights/biases are identical on every core
    wqt = np.ascontiguousarray(Wq.T).astype(bf).reshape(EC, P, E)
    wkt = np.ascontiguousarray(Wk.T).astype(bf).reshape(EC, P, E)
    wvt = np.ascontiguousarray(Wv.T).astype(bf).reshape(EC, P, E)
    wot = np.ascontiguousarray(Wo.T).astype(bf).reshape(EC, P, E)
    bq_a = np.ascontiguousarray(bq).reshape(HP, P)
    bk_a = np.ascontiguousarray(bk).reshape(HP, P)
    bvb = np.ascontiguousarray(np.broadcast_to(bv[None, :], (P, E)))
    bob = np.ascontiguousarray(np.broadcast_to(bo[None, :], (P, E)))
    shared = dict(wqt=wqt, wkt=wkt, wvt=wvt, wot=wot, bq=bq_a, bk=bk_a,
                  bvb=bvb, bob=bob, ones=ones_a)

    in_maps = []
    for c in range(NCORES):
        b, qh = c // 2, c % 2
        xb = x[b]
        if qh == 1:
            # own query half first; key order is a permutation, which
            # softmax+PV are invariant to
            xb = np.concatenate([xb[QT:], xb[:QT]], axis=0)
        xt = np.ascontiguousarray(xb.T).astype(bf).reshape(EC, P, T)
        in_maps.append({"xt": xt, **shared})
    return in_maps


def kernel(x, Wq, bq, Wk, bk, Wv, bv, Wo, bo):
    nc = _get_nc()
    in_maps = _prep_core_inputs(x, Wq, bq, Wk, bk, Wv, bv, Wo, bo)
    res = run_bass_kernel_spmd(nc, in_maps, list(range(NCORES)))
    out = np.empty((B, T, E), np.float32)
    for c in range(NCORES):
        b, qh = c // 2, c % 2
        out[b, qh * QT:(qh + 1) * QT, :] = res.results[c]["out"].reshape(QT, E)
    return out


# revision 21
# speedup vs baseline: 3.6760x; 3.6760x over previous
"""MultiHeadAttention Trainium2 kernel (8-core SPMD, collective-free).

Problem: B=4, T=2048, E=1024, H=16, D=64 multi-head self-attention
(torch-style Linear projections, softmax over keys, output projection).

Sharding: core c handles batch b=c//2 and query-token half qh=c%2
(1024 query tokens) with ALL 16 heads local, so the output projection
contracts over the full E=1024 attention features with no cross-core
communication at all (the collective path through fake_nrt costs ~60ms
per AllGather, ~250ms/iter -- removing it is worth the 2x duplicated
K/V projection work).  K and V cover all 2048 keys; each core's xt is
host-side PERMUTED so its own query half comes first (tokens 0:1024) --
softmax is permutation-invariant over keys, so the same SPMD program
works on every core.

Device pipeline per core (storage bf16, all accumulation fp32):
  - bootstrap: q/k projections for the first two head-pairs + the first
    v half inline (PE-dense; ACT has less total work than PE, so idling
    ACT here is free),
  - 16 weave pair-units (8 head-pairs x 2 query blocks of 512), each 17
    kc-steps: step s emits the two heads' score matmuls for kc=s
    (scores^T [128 keys, 512 q], K=64, adjacent at tile_position (0,0)/
    (64,0) so they run concurrently on different PE array row-halves),
    ONE exp ACT [P,2,512] (scale=1/8 fused; logits bounded so no max
    subtraction), then the PV matmuls for kc=s-1 via a 4-slot exp ring,
  - PSUM is partitioned so nothing steals the scores' lookahead:
    scores 2 slots x 2 banks (a full ACT period of lookahead), fillers
    (projections) 2 x 1 bank, PV accumulators 2 x 1 bank,
  - PV lhsT = [v_h | 1]: psum row 64 accumulates the softmax
    denominator for free; po drains to an SBUF f32r staging pair with
    one DVE copy so the PV accumulator frees after one unit; the
    normalize (K=1 ones-matmul partition-broadcast into the FILLER psum
    pool, DVE reciprocal + multiply) runs a unit later, fully decoupled,
  - odd heads' outputs are partition-shifted 0:64 -> 64:128 with a
    small SBUF->SBUF DMA so outT keeps a feature-major layout,
  - remaining projections run as EDF-deadline-ordered filler chunks
    ([P,512] PSUM granularity) in the weave's PE gaps.
"""

import os
import sys
from contextlib import ExitStack

import numpy as np
import ml_dtypes

for _p in ("/opt/trn_rl_repo", "/root/.axon_site/_ro/trn_rl_repo"):
    if os.path.isdir(_p) and _p not in sys.path:
        sys.path.insert(0, _p)

import concourse.bass as bass  # noqa: E402,F401
from concourse import bacc  # noqa: E402
import concourse.tile as tile  # noqa: E402
from concourse import mybir  # noqa: E402
from concourse.bass_utils import run_bass_kernel_spmd  # noqa: E402

# ---- problem constants (hardcoded; kernel.py must be self-contained) ----
B, T, E, H, D = 4, 2048, 1024, 16, 64
P = 128
NCORES = 8
QT = 1024            # local query tokens per core
EC = E // P          # 8 contraction chunks for projections
HP = H // 2          # 8 head-pair feature chunks (128 rows = 2 heads)
KC = T // P          # 16 key-token chunks
QB = QT // 512       # 2 query blocks of 512
TC = QT // P         # 8 output token chunks of 128
NG = KC // 2         # 8 kc-pair score groups per unit

BF = mybir.dt.bfloat16
F32 = mybir.dt.float32
F32R = mybir.dt.float32r
AF = mybir.ActivationFunctionType
ALU = mybir.AluOpType

SECTIONS = []        # (name, first_instruction_index) markers for profiling
USE_TILE_POS = os.environ.get("KERNEL_NO_TILEPOS", "0") != "1"
REPEAT = int(os.environ.get("KERNEL_REPEAT", "1"))
# timing-only ablations (produce wrong outputs; for HW attribution):
#   noexp / nopv / nosc / nofill / nofinish, comma-separated
ABL = set(filter(None, os.environ.get("KERNEL_ABL", "").split(",")))


def build_program():
    nc = bacc.Bacc("TRN2", target_bir_lowering=False, debug=False,
                   num_devices=NCORES)

    def mark(name):
        SECTIONS.append((name, len(nc.inst_map)))

    xt_d = nc.dram_tensor("xt", [EC, P, T], BF, kind="ExternalInput").ap()
    wqt_d = nc.dram_tensor("wqt", [EC, P, E], BF, kind="ExternalInput").ap()
    wkt_d = nc.dram_tensor("wkt", [EC, P, E], BF, kind="ExternalInput").ap()
    wvt_d = nc.dram_tensor("wvt", [EC, P, E], BF, kind="ExternalInput").ap()
    wot_d = nc.dram_tensor("wot", [EC, P, E], BF, kind="ExternalInput").ap()
    bq_d = nc.dram_tensor("bq", [HP, P], F32, kind="ExternalInput").ap()
    bk_d = nc.dram_tensor("bk", [HP, P], F32, kind="ExternalInput").ap()
    bvb_d = nc.dram_tensor("bvb", [P, E], F32, kind="ExternalInput").ap()
    bob_d = nc.dram_tensor("bob", [P, E], F32, kind="ExternalInput").ap()
    out_d = nc.dram_tensor("out", [TC, P, E], F32,
                           kind="ExternalOutput").ap()

    with tile.TileContext(nc) as tc, ExitStack() as ctx:
        persist = ctx.enter_context(tc.tile_pool(name="persist", bufs=1))
        wq_pool = ctx.enter_context(tc.tile_pool(name="wq", bufs=2))
        wv_pool = ctx.enter_context(tc.tile_pool(name="wv", bufs=2))
        ehp = ctx.enter_context(tc.tile_pool(name="ehp", bufs=4))
        small = ctx.enter_context(tc.tile_pool(name="small", bufs=2))
        otmp_pool = ctx.enter_context(tc.tile_pool(name="otmp", bufs=1))
        fin_pool = ctx.enter_context(tc.tile_pool(name="finp", bufs=2))
        stag_pool = ctx.enter_context(tc.tile_pool(name="stag", bufs=3))
        psc = ctx.enter_context(tc.tile_pool(name="psc", bufs=2, space="PSUM"))
        pfl = ctx.enter_context(tc.tile_pool(name="pfl", bufs=2, space="PSUM"))
        ppv = ctx.enter_context(tc.tile_pool(name="ppv", bufs=2, space="PSUM"))

        def sc_slot():
            return psc.tile([P, 2, 512], F32, tag="sc", name="sc")

        # ---------------- persistent SBUF tensors ----------------
        xt_sb = persist.tile([P, EC, T], BF, tag="xt")          # 32K
        bq_sb = persist.tile([P, HP], F32, tag="bq")
        bk_sb = persist.tile([P, HP], F32, tag="bk")
        bvb_sb = persist.tile([P, E], F32, tag="bvb")           # 4K
        bob_sb = persist.tile([P, E], F32, tag="bob")           # 4K
        wot_sb = persist.tile([P, EC, E], BF, tag="wot")        # 16K
        qt_sb = persist.tile([P, HP, QT], BF, tag="qt")         # 16K
        kt_sb = persist.tile([P, HP, T], BF, tag="kt")          # 32K
        vaug = persist.tile([P, KC, H * 65], BF, tag="vaug")    # 32.5K
        outT = persist.tile([P, HP, QT], BF, tag="outT")        # 16K

        mark('in_dma')
        # ---------------- input DMAs ----------------
        for ec in range(EC):
            nc.sync.dma_start(xt_sb[:, ec, 0:1024], xt_d[ec, :, 0:1024])
        nc.sync.dma_start(bq_sb[:], bq_d.rearrange("f p -> p f"))
        nc.sync.dma_start(bk_sb[:], bk_d.rearrange("f p -> p f"))
        nc.sync.dma_start(bvb_sb[:], bvb_d)
        for ec in range(EC):
            nc.sync.dma_start(xt_sb[:, ec, 1024:T], xt_d[ec, :, 1024:T])
        ones_sb = persist.tile([P, 64], F32R, tag="ones")
        ones_f = persist.tile([P, 64], F32, tag="onesf")
        nc.gpsimd.memset(ones_f[:], 1.0)
        with nc.allow_low_precision(reason="f32r is f32 storage"):
            nc.vector.tensor_copy(ones_sb[:], ones_f[:])
        vaug_h = vaug.rearrange("p k (h c) -> p k h c", c=65)
        for h in range(H):
            nc.gpsimd.memset(vaug_h[:, :, h, 64:65], 1.0)

        # ablation dummies: memset-once stand-ins for skipped producers
        eh_fix = dum_sb = None
        if "noexp" in ABL:
            eh_fix = [persist.tile([P, 2, 512], BF, tag=f"ehfix{i}",
                                   name=f"ehfix{i}") for i in range(4)]
            for t_ in eh_fix:
                nc.gpsimd.memset(t_[:], 1.0)
        if "nosc" in ABL:
            dum_sb = persist.tile([P, 2, 512], F32, tag="dumsb")
            nc.gpsimd.memset(dum_sb[:], 1.0)
        if "nofill" in ABL:
            nc.gpsimd.memset(qt_sb[:], 1.0)
            nc.gpsimd.memset(kt_sb[:], 1.0)
            nc.gpsimd.memset(vaug_h[:, :, :, 0:64], 0.001)
        if "nopv" in ABL or "nofinish" in ABL:
            nc.gpsimd.memset(outT[:], 1.0)

        # pair-units: two heads of a head-pair chunk per unit, so their
        # K=64 score matmuls land on different PE array row-halves and
        # run concurrently; query-block major so each block's output
        # projection can start as early as possible
        UNITS = [(hp, qb) for qb in range(QB) for hp in range(HP)]

        # EDF filler queue: (deadline unit, seq, fn) -- drained in
        # deadline order so long low-urgency batches (v pass 1) cannot
        # starve imminent q/k chunks
        filler = []
        fseq = [0]

        def queue_chunk(dl, fn):
            filler.append((dl, fseq[0], fn))
            fseq[0] += 1
            filler.sort()

        def drain_filler(n=1):
            for _ in range(n):
                if not filler:
                    return
                filler.pop(0)[2]()

        def fl_slot():
            return pfl.tile([P, 512], F32, tag="fl", name="fl")

        def queue_qk(fc, dl):
            """qT/kT projection for feature chunk fc (head pair) as
            fillers; [P,512] token-block granularity on the filler pool."""
            fs = slice(fc * P, (fc + 1) * P)
            box = {}

            def dma_w():
                box["wqf"] = wq_pool.tile([P, EC, P], BF, tag="wqf",
                                          name="wqf")
                box["wkf"] = wq_pool.tile([P, EC, P], BF, tag="wkf",
                                          name="wkf")
                for ec in range(EC):
                    nc.sync.dma_start(box["wqf"][:, ec, :], wqt_d[ec, :, fs])
                    nc.sync.dma_start(box["wkf"][:, ec, :], wkt_d[ec, :, fs])
            queue_chunk(dl, dma_w)

            def group(kind, tb):
                w_key = "wqf" if kind == "q" else "wkf"
                dest = qt_sb if kind == "q" else kt_sb
                bias = bq_sb if kind == "q" else bk_sb
                ps_box = {}

                def mms(lo, hi):
                    def _f():
                        if "ps" not in ps_box:
                            ps_box["ps"] = fl_slot()
                        for ec in range(lo, hi):
                            nc.tensor.matmul(
                                ps_box["ps"][:], lhsT=box[w_key][:, ec, :],
                                rhs=xt_sb[:, ec, tb * 512:(tb + 1) * 512],
                                start=(ec == 0), stop=(ec == EC - 1),
                            )
                    return _f
                queue_chunk(dl, mms(0, 4))
                queue_chunk(dl, mms(4, 8))

                def evac():
                    nc.vector.tensor_scalar_add(
                        dest[:, fc, tb * 512:(tb + 1) * 512],
                        ps_box["ps"][:], bias[:, fc: fc + 1],
                    )
                queue_chunk(dl, evac)
            group("q", 0)                    # q: 1024 local query tokens
            group("q", 1)
            for tb in range(T // 512):       # k: all 2048 key tokens
                group("k", tb)

        def queue_outproj(qb, dl):
            """final[t, :] for the 4 token chunks of query block qb."""
            for tcl in range(4):
                for half in range(2):
                    tc_ = qb * 4 + tcl
                    ps_box = {}

                    def mms(lo, hi, tc_=tc_, half=half, ps_box=ps_box):
                        def _f():
                            if "ps" not in ps_box:
                                ps_box["ps"] = fl_slot()
                            for fc in range(lo, hi):
                                nc.tensor.matmul(
                                    ps_box["ps"][:],
                                    lhsT=outT[:, fc, tc_ * P:(tc_ + 1) * P],
                                    rhs=wot_sb[:, fc,
                                               half * 512:(half + 1) * 512],
                                    start=(fc == 0), stop=(fc == EC - 1),
                                )
                        return _f
                    queue_chunk(dl, mms(0, 4))
                    queue_chunk(dl, mms(4, 8))

                    def evac(tc_=tc_, half=half, ps_box=ps_box):
                        fin = fin_pool.tile([P, 512], F32, tag="fin",
                                            name="fin")
                        nc.vector.tensor_tensor(
                            fin[:], ps_box["ps"][:],
                            bob_sb[:, half * 512:(half + 1) * 512], ALU.add)
                        nc.sync.dma_start(
                            out_d[tc_][:, half * 512:(half + 1) * 512],
                            fin[:])
                    queue_chunk(dl, evac)

        def queue_vproj(hf, dl):
            """v projection pass for feature half hf (heads hf*8..+8); the
            hf=0 pass runs inline in the bootstrap."""
            bvb_v = bvb_sb.rearrange("p (h d) -> p h d", d=D)
            box = {}

            def dma_w():
                box["wvh"] = wv_pool.tile([P, EC, 512], BF, tag="wvh",
                                          name="wvh")
                for ec in range(EC):
                    nc.sync.dma_start(box["wvh"][:, ec, :],
                                      wvt_d[ec, :, hf * 512:(hf + 1) * 512])
            if hf > 0:
                queue_chunk(dl, dma_w)
            else:
                dma_w()

            def emit_kc(kc):
                ps_box = {}

                def mms(lo, hi):
                    def _f():
                        if "ps" not in ps_box:
                            ps_box["ps"] = fl_slot()
                        for ec in range(lo, hi):
                            nc.tensor.matmul(
                                ps_box["ps"][:],
                                lhsT=xt_sb[:, ec, kc * P:(kc + 1) * P],
                                rhs=box["wvh"][:, ec, :],
                                start=(ec == lo and lo == 0),
                                stop=(ec == EC - 1),
                            )
                    return _f

                def evac():
                    nc.vector.tensor_tensor(
                        vaug_h[:, kc, hf * 8:(hf + 1) * 8, 0:64],
                        ps_box["ps"].rearrange("p (h d) -> p h d", d=D),
                        bvb_v[:, hf * 8:(hf + 1) * 8, :],
                        ALU.add,
                    )
                return [mms(0, 4), mms(4, 8), evac]

            for kc in range(KC):
                chunks = emit_kc(kc)
                if hf > 0:
                    for ch in chunks:
                        queue_chunk(dl, ch)
                else:
                    for ch in chunks:
                        ch()
                    drain_filler(1)

        def normalize(hp, par, qb, stag):
            """Softmax normalize from the SBUF staging tile: row 64
            holds the denominator.  A K=1 float32r ones-matmul broadcasts
            it across partitions INTO THE FILLER PSUM POOL (never the
            scores rotation), reading the stag row copied a full unit
            earlier so the PE never waits on DVE here; then DVE
            reciprocal + multiply."""
            qs = slice(qb * 512, (qb + 1) * 512)
            srb = small.tile([P, 512], F32R, tag="srb", name="srb")
            psR = fl_slot()
            nc.tensor.matmul(psR[0:64, :], lhsT=ones_sb[64:65, :],
                             rhs=stag[64:65, :], start=True, stop=True)
            with nc.allow_low_precision(
                    reason="float32r is bit-identical fp32 storage"):
                nc.vector.reciprocal(srb[0:64, :], psR[0:64, :])
            if par == 0:
                nc.vector.tensor_tensor(outT[0:64, hp, qs], stag[0:64, :],
                                        srb[0:64, :], ALU.mult)
            else:
                ot = otmp_pool.tile([P, 512], BF, tag="ot", name="ot")
                nc.vector.tensor_tensor(ot[0:64, :], stag[0:64, :],
                                        srb[0:64, :], ALU.mult)
                nc.sync.dma_start(outT[64:128, hp, qs], ot[0:64, :])

        finish_box = {}  # previous unit's (hp, qb, stag-pair)

        def do_finish():
            if finish_box:
                hp, qb, st2 = finish_box.pop("prev")
                normalize(hp, 0, qb, st2[0])
                normalize(hp, 1, qb, st2[1])

        def weave_unit(ui, budget):
            """Emit pair-unit ui (heads 2hp, 2hp+1): 17 kc-steps; step s
            emits the two heads' score matmuls for kc=s (adjacent, on
            different array row-halves -> concurrent) + ONE exp ACT
            [P,2,512], then the PV matmuls for kc=s-1 via the 4-slot exp
            ring.  One [P,2,512] PSUM slot per kc (bufs=2) gives scores a
            full ACT period of lookahead; fillers live on their own PSUM
            pool so they never steal that lookahead.  po drains to an
            SBUF staging pair with one DVE copy; normalization runs fully
            decoupled at the next unit's start."""
            hp, qb = UNITS[ui]
            qs = slice(qb * 512, (qb + 1) * 512)
            rows2 = (slice(0, 64), slice(64, 128))
            tp2 = (dict(tile_position=(0, 0)), dict(tile_position=(64, 0))) \
                if USE_TILE_POS else ({}, {})
            po2 = [ppv.tile([P, 512], F32, tag="po", name="po")
                   for _ in range(2)]
            do_finish()
            ring = [None] * 4
            for s in range(KC + 1):
                if s < KC:
                    kc = s
                    kslc = slice(kc * P, (kc + 1) * P)
                    ps = None
                    if "nosc" not in ABL:
                        ps = sc_slot()
                        for par in range(2):
                            nc.tensor.matmul(
                                ps[:, par, :],
                                lhsT=kt_sb[rows2[par], hp, kslc],
                                rhs=qt_sb[rows2[par], hp, qs],
                                start=True, stop=True, **tp2[par],
                            )
                    if "noexp" in ABL:
                        ring[s % 4] = eh_fix[s % 4]
                    else:
                        eh = ehp.tile([P, 2, 512], BF, tag="eh", name="eh")
                        nc.scalar.activation(
                            eh[:], dum_sb[:] if "nosc" in ABL else ps[:],
                            AF.Exp, scale=0.125)
                        ring[s % 4] = eh
                if s >= 1 and "nopv" not in ABL:
                    kc = s - 1
                    for par in range(2):
                        nc.tensor.matmul(
                            po2[par][0:65, :],
                            lhsT=vaug_h[:, kc, 2 * hp + par, :],
                            rhs=ring[kc % 4][:, par, :],
                            start=(kc == 0), stop=(kc == KC - 1),
                        )
                drain_filler(budget)
            if "nopv" not in ABL and "nofinish" not in ABL:
                st2 = [stag_pool.tile([P, 512], F32R, tag="stag",
                                      name="stag")
                       for _ in range(2)]
                for par in range(2):
                    nc.vector.tensor_copy(st2[par][0:65, :],
                                          po2[par][0:65, :])
                finish_box["prev"] = (hp, qb, st2)

        def emit_body():
            mark('proj')
            # bootstrap: q/k projections for the first two chunks + the
            # first v half inline -- PE-dense, ACT idle (ACT has spare
            # total budget, so idling it here is free)
            if "nofill" not in ABL:
                queue_qk(0, 0)
                queue_qk(1, 1)
                queue_vproj(0, 0)   # inline; drains 16 qk chunks
                drain_filler(3)     # rest of qk(0); qk(1) drains in unit 0

            mark('attention')
            for ec in range(EC):
                nc.sync.dma_start(wot_sb[:, ec, :], wot_d[ec])
            nc.sync.dma_start(bob_sb[:], bob_d)

            # EDF deadlines: qk(fc) before unit fc, v half 1 before unit 4
            # (first PV of heads 8/9), outproj(qb0) after unit 8's finish
            for ui in range(len(UNITS)):
                if "nofill" not in ABL:
                    if ui == 0:
                        queue_qk(2, 2)
                        queue_qk(3, 3)
                    if ui == 1:
                        queue_qk(4, 4)
                        queue_qk(5, 5)
                    if ui == 2:
                        queue_vproj(1, 4)
                        queue_qk(6, 6)
                    if ui == 3:
                        queue_qk(7, 7)
                    if ui == HP + 1:            # outT[:, :, qb0] complete
                        queue_outproj(0, ui)
                weave_unit(ui, budget=2 if ui <= 5 else 1)
            do_finish()
            if "nofill" not in ABL:
                queue_outproj(QB - 1, 99)
            drain_filler(300)
            do_finish()

        for _rep in range(REPEAT):
            emit_body()

        mark('tail')
    nc.compile()
    return nc


_NC = None


def _get_nc():
    global _NC
    if _NC is None:
        _NC = build_program()
    return _NC


def _prep_core_inputs(x, Wq, bq, Wk, bk, Wv, bv, Wo, bo):
    """Build the 8 per-core input dicts (host-side sharding)."""
    bf = ml_dtypes.bfloat16
    x = np.asarray(x, dtype=np.float32)
    Wq, Wk, Wv, Wo = (np.asarray(a, np.float32) for a in (Wq, Wk, Wv, Wo))
    bq, bk, bv, bo = (np.asarray(a, np.float32) for a in (bq, bk, bv, bo))

    # weights/biases are identical on every core
    wqt = np.ascontiguousarray(Wq.T).astype(bf).reshape(EC, P, E)
    wkt = np.ascontiguousarray(Wk.T).astype(bf).reshape(EC, P, E)
    wvt = np.ascontiguousarray(Wv.T).astype(bf).reshape(EC, P, E)
    wot = np.ascontiguousarray(Wo.T).astype(bf).reshape(EC, P, E)
    bq_a = np.ascontiguousarray(bq).reshape(HP, P)
    bk_a = np.ascontiguousarray(bk).reshape(HP, P)
    bvb = np.ascontiguousarray(np.broadcast_to(bv[None, :], (P, E)))
    bob = np.ascontiguousarray(np.broadcast_to(bo[None, :], (P, E)))
    shared = dict(wqt=wqt, wkt=wkt, wvt=wvt, wot=wot, bq=bq_a, bk=bk_a,
                  bvb=bvb, bob=bob)

    in_maps = []
    for c in range(NCORES):
        b, qh = c // 2, c % 2
        xb = x[b]
        if qh == 1:
            # own query half first; key order is a permutation, which
            # softmax+PV are invariant to
            xb = np.concatenate([xb[QT:], xb[:QT]], axis=0)
        xt = np.ascontiguousarray(xb.T).astype(bf).reshape(EC, P, T)
        in_maps.append({"xt": xt, **shared})
    return in_maps


def kernel(x, Wq, bq, Wk, bk, Wv, bv, Wo, bo):
    nc = _get_nc()
    in_maps = _prep_core_inputs(x, Wq, bq, Wk, bk, Wv, bv, Wo, bo)
    res = run_bass_kernel_spmd(nc, in_maps, list(range(NCORES)))
    out = np.empty((B, T, E), np.float32)
    for c in range(NCORES):
        b, qh = c // 2, c % 2
        out[b, qh * QT:(qh + 1) * QT, :] = res.results[c]["out"].reshape(QT, E)
    return out
